# revision 13
# baseline (speedup 1.0000x reference)
"""Routed low-rank FFN (MoE-style) Trainium2 kernel.

out[n] = x[n] @ U[pids[n]] @ V[pids[n]] + bias

Strategy (expert-parallel over 8 NeuronCores):
  - Host: stable-sort tokens by pid; expert p's tokens go to core p // 8.
    Each expert's token list is split into chunks of <= 128 tokens
    ("groups"); every core runs the same static program over G groups of
    capacity C (zero-padded), so the SPMD program is identical on all
    cores while the data differs.
  - Reduced-precision I/O (the kernel is DMA-bound; harness gate is
    rel_err < 2e-2, measured ~8e-3 for this mix):
      x, U  -> fp8 e4m3 (packed into one DRAM tensor "xu" per core)
      V, bias, h, out -> fp16
  - Device, per group g (one expert's <=C tokens):
      h^T [64, C]    = sum_k U_chunk[k].T @ x_chunk[k]  (8 fp8 matmuls, K=128)
      out [C, 1024]  = [h^T; ones].T @ [V; bias]        (2 fp16 matmuls, N=512)
    The ones row folds the bias add into the second matmul; PSUM keeps
    f32 accuracy throughout.
  - DMA: xu streams on the sync HWDGE queue, vb on the scalar HWDGE
    queue (no SWDGE - its Q7 descriptor generation is ~2us per DMA).
    Outputs are stored as fp16 in pairs of groups (one DMA per 2 groups)
    alternating between the two HWDGE queues.
  - Host: inverse-permute rows back to original token order, cast f32.
"""

import os

import numpy as np
import ml_dtypes

N_CORES = 8
D_IN = 1024
RANK = 64
D_OUT = 1024
KC = 8  # number of 128-deep contraction chunks: D_IN // 128
MAX_CHUNK = 128  # max tokens per group (PE partition limit for matmul 2)

F8 = ml_dtypes.float8_e4m3
F16 = np.float16

# Set by kernel() after a traced run (KERNEL_TRACE=1): HW kernel span in ns.
LAST_EXEC_TIME_NS = None
LAST_RESULTS = None

_PROGRAM_CACHE = {}


def _slice_bounds(G, n_slices):
    """First slice small so compute starts early, rest even."""
    n_slices = max(1, min(n_slices, G))
    if n_slices == 1:
        return [0, G]
    rest = G - 1
    k = n_slices - 1
    bounds = [0, 1]
    for i in range(1, k + 1):
        bounds.append(1 + round(i * rest / k))
    return bounds


def _build_program(G: int, C: int):
    """Build the SPMD Bass/Tile program: G groups of capacity C per core."""
    import concourse.tile as tile
    from concourse import bacc, mybir

    nc = bacc.Bacc(
        "TRN2",
        target_bir_lowering=False,
        debug=False,
        enable_asserts=False,
        num_devices=N_CORES,
    )
    f32 = mybir.dt.float32
    f16 = mybir.dt.float16
    f8 = mybir.dt.float8e4

    FW = KC * C + KC * RANK  # flat per-(partition, group) row: x chunks + u chunks
    XOFF = 0
    UOFF = KC * C

    xu_d = nc.dram_tensor("xug", [128, G, FW], f8, kind="ExternalInput")
    vb_d = nc.dram_tensor("vbg", [RANK + 1, G, D_OUT], f16, kind="ExternalInput")
    o_d = nc.dram_tensor("og", [G, C, D_OUT], f16, kind="ExternalOutput")

    n2 = D_OUT // 512  # matmul-2 free-dim splits (one PSUM bank each)

    xbounds = _slice_bounds(G, 4)  # xu slices (sync queue)
    vbounds = _slice_bounds(G, 3)  # vb slices (scalar queue, slower emission)

    with tile.TileContext(nc) as tc:
        with (
            tc.tile_pool(name="xin", bufs=1) as xpool,
            tc.tile_pool(name="win", bufs=1) as wpool,
            tc.tile_pool(name="hbuf", bufs=1) as hpool,
            tc.tile_pool(name="obuf", bufs=3) as opool,
            tc.tile_pool(name="ph", bufs=1, space="PSUM") as phpool,
            tc.tile_pool(name="po", bufs=1, space="PSUM") as popool,
            tc.tile_pool(name="wm", bufs=1, space="PSUM") as wmpool,
        ):
            # f32 ones row; the [hT; ones] trick folds the bias matmul row.
            ones_sb = wpool.tile([1, C], f32, tag="ones")
            nc.vector.memset(ones_sb[:], 1.0)

            # Two hT buffers with the fp16 ones row prefilled once.
            hTs = [
                hpool.tile([RANK + 1, C], f16, tag=f"h{i}", name=f"hT{i}")
                for i in range(2)
            ]
            for i in range(2):
                nc.vector.tensor_copy(hTs[i][RANK : RANK + 1, :], ones_sb[:])

            # HAM warmup: dense back-to-back N=512 fp16 matmuls (~86% PE
            # duty for >4us) while the first input slices stream in, so
            # real matmuls run at 2.4 GHz, not the throttled 1.2.
            wm_lhs = wpool.tile([RANK + 1, C], f16, tag="wml")
            wm_rhs = wpool.tile([RANK + 1, 512], f16, tag="wmr")
            nc.vector.memset(wm_lhs[:], 0.0)
            nc.vector.memset(wm_rhs[:], 0.0)
            wm_ps = wmpool.tile([C, 512], f32, tag="wm")
            for _ in range(8):
                nc.tensor.matmul(
                    wm_ps[:], lhsT=wm_lhs[:], rhs=wm_rhs[:], start=True, stop=True
                )

            xu_parts, vb_parts = [], []
            for s in range(len(xbounds) - 1):
                g0, g1 = xbounds[s], xbounds[s + 1]
                xu_sb = xpool.tile([128, g1 - g0, FW], f8, tag=f"xu{s}")
                nc.sync.dma_start(out=xu_sb[:], in_=xu_d[:, g0:g1])
                xu_parts.append(xu_sb)
            for s in range(len(vbounds) - 1):
                g0, g1 = vbounds[s], vbounds[s + 1]
                vb_sb = wpool.tile([RANK + 1, g1 - g0, D_OUT], f16, tag=f"vb{s}")
                nc.scalar.dma_start(out=vb_sb[:], in_=vb_d[:, g0:g1])
                vb_parts.append(vb_sb)

            def slice_of(g):
                sx = next(i for i in range(len(xbounds) - 1) if xbounds[i + 1] > g)
                sv = next(i for i in range(len(vbounds) - 1) if vbounds[i + 1] > g)
                return (
                    xu_parts[sx],
                    g - xbounds[sx],
                    vb_parts[sv],
                    g - vbounds[sv],
                )

            # Software pipeline over groups: issue mm1(g) before mm2(g-1)
            # so the PE never waits on the DVE hT cast; epilogue copies are
            # split across ScalarE/VectorE halves to stay off the critical
            # path; one output store per group on the sync queue.
            phs, pos = [None] * G, [None] * G

            def mm1(g):
                xu_sb, gl, _, _ = slice_of(g)
                ph = phpool.tile([RANK, C], f32, tag=f"ph{g % 2}")
                phs[g] = ph
                for k in range(KC):
                    nc.tensor.matmul(
                        ph[:],
                        lhsT=xu_sb[:, gl, UOFF + k * RANK : UOFF + (k + 1) * RANK],
                        rhs=xu_sb[:, gl, XOFF + k * C : XOFF + (k + 1) * C],
                        start=(k == 0),
                        stop=(k == KC - 1),
                    )

            def hcast(g):
                # fp16 rounding of h^T; ones row is prefilled
                nc.vector.tensor_copy(hTs[g % 2][0:RANK, :], phs[g][:])

            def mm2(g):
                _, _, vb_sb, gv = slice_of(g)
                po = popool.tile([C, D_OUT], f32, tag=f"po{g % 2}")
                pos[g] = po
                for j in range(n2):
                    nc.tensor.matmul(
                        po[:, j * 512 : (j + 1) * 512],
                        lhsT=hTs[g % 2][:],
                        rhs=vb_sb[:, gv, j * 512 : (j + 1) * 512],
                        start=True,
                        stop=True,
                    )

            def epilogue_store(g):
                po = pos[g]
                o_sb = opool.tile([C, D_OUT], f16, tag="o")
                half = D_OUT // 2
                nc.scalar.copy(o_sb[:, 0:half], po[:, 0:half])
                nc.vector.tensor_copy(o_sb[:, half:], po[:, half:])
                nc.sync.dma_start(out=o_d[g], in_=o_sb[:])

            mm1(0)
            hcast(0)
            for g in range(1, G):
                mm1(g)
                mm2(g - 1)
                hcast(g)
                epilogue_store(g - 1)
            mm2(G - 1)
            epilogue_store(G - 1)

    nc.compile()
    return nc


def _route(pids: np.ndarray, n_experts: int):
    """Group token indices by expert, chunk to MAX_CHUNK, assign to cores."""
    order = np.argsort(pids, kind="stable")
    counts = np.bincount(pids, minlength=n_experts)
    per_core = n_experts // N_CORES
    core_groups = [[] for _ in range(N_CORES)]
    off = 0
    for p in range(n_experts):
        toks = order[off : off + counts[p]]
        off += counts[p]
        for s in range(0, len(toks), MAX_CHUNK):
            core_groups[p // per_core].append((p, toks[s : s + MAX_CHUNK]))
    return core_groups


def _capacity(core_groups):
    G = max(len(gs) for gs in core_groups)
    maxlen = max((len(t) for gs in core_groups for _, t in gs), default=1)
    C = int(min(MAX_CHUNK, max(16, 4 * -(-maxlen // 4))))
    return G, C


def _pack_core(groups, G, C, x8, U8, V16, bias16):
    """Build one core's in_map from its (pid, token) groups."""
    FW = KC * C + KC * RANK
    xu = np.zeros((128, G, FW), F8)
    vb = np.zeros((RANK + 1, G, D_OUT), F16)
    for gi, (p, toks) in enumerate(groups):
        blk = np.zeros((C, D_IN), F8)
        blk[: len(toks)] = x8[toks]
        # [C, D] -> [d, t] -> [k, dp, t] -> [dp, k, t] -> flat [dp, k*t]
        xu[:, gi, : KC * C] = (
            blk.T.reshape(KC, 128, C).transpose(1, 0, 2).reshape(128, KC * C)
        )
        xu[:, gi, KC * C :] = (
            U8[p].reshape(KC, 128, RANK).transpose(1, 0, 2).reshape(128, KC * RANK)
        )
        vb[:RANK, gi] = V16[p]
        vb[RANK, gi] = bias16
    return {"xug": xu, "vbg": vb}


def _unpack(og_list, core_groups, N):
    out = np.zeros((N, D_OUT), np.float32)
    for c in range(N_CORES):
        og = np.asarray(og_list[c]).astype(np.float32)
        for gi, (p, toks) in enumerate(core_groups[c]):
            out[toks] = og[gi, : len(toks)]
    return out


def kernel(x, pids, U, V, bias):
    global LAST_EXEC_TIME_NS, LAST_RESULTS
    from concourse.bass_utils import run_bass_kernel_spmd

    x = np.asarray(x, dtype=np.float32)
    pids_np = np.asarray(pids).astype(np.int64)
    U = np.asarray(U, dtype=np.float32)
    V = np.asarray(V, dtype=np.float32)
    bias = np.asarray(bias, dtype=np.float32)

    N = x.shape[0]
    P = U.shape[0]

    x8 = x.astype(F8)
    U8 = U.astype(F8)
    V16 = V.astype(F16)
    bias16 = bias.astype(F16)

    core_groups = _route(pids_np, P)
    G, C = _capacity(core_groups)

    in_maps = [
        _pack_core(core_groups[c], G, C, x8, U8, V16, bias16)
        for c in range(N_CORES)
    ]

    key = (G, C)
    if key not in _PROGRAM_CACHE:
        _PROGRAM_CACHE[key] = _build_program(G, C)
    nc = _PROGRAM_CACHE[key]

    trace = os.environ.get("KERNEL_TRACE", "0") == "1"
    res = run_bass_kernel_spmd(nc, in_maps, list(range(N_CORES)), trace=trace)
    LAST_EXEC_TIME_NS = res.exec_time_ns
    LAST_RESULTS = res

    return _unpack([res.results[c]["og"] for c in range(N_CORES)], core_groups, N)


# revision 16
# speedup vs baseline: 1.0198x; 1.0198x over previous
"""Routed low-rank FFN (MoE-style) Trainium2 kernel.

out[n] = x[n] @ U[pids[n]] @ V[pids[n]] + bias

Strategy (expert-parallel over 8 NeuronCores):
  - Host: stable-sort tokens by pid; expert p's tokens go to core p // 8.
    Each expert's token list is split into chunks of <= 128 tokens
    ("groups"); every core runs the same static program over G groups of
    capacity C (zero-padded), so the SPMD program is identical on all
    cores while the data differs.
  - Reduced-precision I/O (the kernel is DMA-bound; harness gate is
    rel_err < 2e-2, measured ~8e-3 for this mix):
      x, U  -> fp8 e4m3 (packed into one DRAM tensor "xu" per core)
      V, bias, h, out -> fp16
  - Device, per group g (one expert's <=C tokens):
      h^T [64, C]    = sum_k U_chunk[k].T @ x_chunk[k]  (8 fp8 matmuls, K=128)
      out [C, 1024]  = [h^T; ones].T @ [V; bias]        (2 fp16 matmuls, N=512)
    The ones row folds the bias add into the second matmul; PSUM keeps
    f32 accuracy throughout.
  - DMA: xu streams on the sync HWDGE queue, vb on the scalar HWDGE
    queue (no SWDGE - its Q7 descriptor generation is ~2us per DMA).
    Outputs are stored as fp16 in pairs of groups (one DMA per 2 groups)
    alternating between the two HWDGE queues.
  - Host: inverse-permute rows back to original token order, cast f32.
"""

import os

import numpy as np
import ml_dtypes

N_CORES = 8
D_IN = 1024
RANK = 64
D_OUT = 1024
KC = 8  # number of 128-deep contraction chunks: D_IN // 128
MAX_CHUNK = 128  # max tokens per group (PE partition limit for matmul 2)

F8 = ml_dtypes.float8_e4m3
F16 = np.float16

# Set by kernel() after a traced run (KERNEL_TRACE=1): HW kernel span in ns.
LAST_EXEC_TIME_NS = None
LAST_RESULTS = None

_PROGRAM_CACHE = {}


def _slice_bounds(G, n_slices):
    """First slice small so compute starts early, rest even."""
    n_slices = max(1, min(n_slices, G))
    if n_slices == 1:
        return [0, G]
    rest = G - 1
    k = n_slices - 1
    bounds = [0, 1]
    for i in range(1, k + 1):
        bounds.append(1 + round(i * rest / k))
    return bounds


def _build_program(G: int, C: int):
    """Build the SPMD Bass/Tile program: G groups of capacity C per core."""
    import concourse.tile as tile
    from concourse import bacc, mybir

    nc = bacc.Bacc(
        "TRN2",
        target_bir_lowering=False,
        debug=False,
        enable_asserts=False,
        num_devices=N_CORES,
    )
    f32 = mybir.dt.float32
    f16 = mybir.dt.float16
    f8 = mybir.dt.float8e4

    FW = KC * C + KC * RANK  # flat per-(partition, group) row: x chunks + u chunks
    XOFF = 0
    UOFF = KC * C

    xu_d = nc.dram_tensor("xug", [128, G, FW], f8, kind="ExternalInput")
    vb_d = nc.dram_tensor("vbg", [RANK + 1, G, D_OUT], f16, kind="ExternalInput")
    o_d = nc.dram_tensor("og", [G, C, D_OUT], f16, kind="ExternalOutput")

    n2 = D_OUT // 512  # matmul-2 free-dim splits (one PSUM bank each)

    xbounds = _slice_bounds(G, 4)  # xu slices (sync queue)
    vbounds = _slice_bounds(G, 3)  # vb slices (scalar queue, slower emission)

    with tile.TileContext(nc) as tc:
        with (
            tc.tile_pool(name="xin", bufs=1) as xpool,
            tc.tile_pool(name="win", bufs=1) as wpool,
            tc.tile_pool(name="hbuf", bufs=1) as hpool,
            tc.tile_pool(name="obuf", bufs=3) as opool,
            tc.tile_pool(name="ph", bufs=1, space="PSUM") as phpool,
            tc.tile_pool(name="po", bufs=1, space="PSUM") as popool,
            tc.tile_pool(name="wm", bufs=1, space="PSUM") as wmpool,
        ):
            # f32 ones row; the [hT; ones] trick folds the bias matmul row.
            ones_sb = wpool.tile([1, C], f32, tag="ones")
            nc.vector.memset(ones_sb[:], 1.0)

            # Three hT buffers with the fp16 ones row prefilled once.
            hTs = [
                hpool.tile([RANK + 1, C], f16, tag=f"h{i}", name=f"hT{i}")
                for i in range(3)
            ]
            for i in range(3):
                nc.vector.tensor_copy(hTs[i][RANK : RANK + 1, :], ones_sb[:])

            # HAM warmup: dense back-to-back N=512 fp16 matmuls (M=32 so
            # the per-matmul LDWEIGHTS is tiny: ~94% PE duty for >3.5us)
            # while the first input slices stream in, so real matmuls run
            # at 2.4 GHz, not the throttled 1.2.
            wm_lhs = wpool.tile([RANK + 1, 32], f16, tag="wml")
            wm_rhs = wpool.tile([RANK + 1, 512], f16, tag="wmr")
            nc.vector.memset(wm_lhs[:], 0.0)
            nc.vector.memset(wm_rhs[:], 0.0)
            wm_ps = wmpool.tile([32, 512], f32, tag="wm")
            for _ in range(8):
                nc.tensor.matmul(
                    wm_ps[:], lhsT=wm_lhs[:], rhs=wm_rhs[:], start=True, stop=True
                )

            xu_parts, vb_parts = [], []
            for s in range(len(xbounds) - 1):
                g0, g1 = xbounds[s], xbounds[s + 1]
                xu_sb = xpool.tile([128, g1 - g0, FW], f8, tag=f"xu{s}")
                nc.sync.dma_start(out=xu_sb[:], in_=xu_d[:, g0:g1])
                xu_parts.append(xu_sb)
            for s in range(len(vbounds) - 1):
                g0, g1 = vbounds[s], vbounds[s + 1]
                vb_sb = wpool.tile([RANK + 1, g1 - g0, D_OUT], f16, tag=f"vb{s}")
                nc.scalar.dma_start(out=vb_sb[:], in_=vb_d[:, g0:g1])
                vb_parts.append(vb_sb)

            def slice_of(g):
                sx = next(i for i in range(len(xbounds) - 1) if xbounds[i + 1] > g)
                sv = next(i for i in range(len(vbounds) - 1) if vbounds[i + 1] > g)
                return (
                    xu_parts[sx],
                    g - xbounds[sx],
                    vb_parts[sv],
                    g - vbounds[sv],
                )

            # Software pipeline over groups: issue mm1(g) before mm2(g-1)
            # so the PE never waits on the DVE hT cast; epilogue copies are
            # split across ScalarE/VectorE halves to stay off the critical
            # path; one output store per group on the sync queue.
            phs, pos = [None] * G, [None] * G

            def mm1(g):
                xu_sb, gl, _, _ = slice_of(g)
                ph = phpool.tile([RANK, C], f32, tag=f"ph{g % 2}")
                phs[g] = ph
                for k in range(KC):
                    nc.tensor.matmul(
                        ph[:],
                        lhsT=xu_sb[:, gl, UOFF + k * RANK : UOFF + (k + 1) * RANK],
                        rhs=xu_sb[:, gl, XOFF + k * C : XOFF + (k + 1) * C],
                        start=(k == 0),
                        stop=(k == KC - 1),
                    )

            def hcast(g):
                # fp16 rounding of h^T; ones row is prefilled
                nc.vector.tensor_copy(hTs[g % 3][0:RANK, :], phs[g][:])

            def mm2(g):
                _, _, vb_sb, gv = slice_of(g)
                po = popool.tile([C, D_OUT], f32, tag=f"po{g % 2}")
                pos[g] = po
                for j in range(n2):
                    nc.tensor.matmul(
                        po[:, j * 512 : (j + 1) * 512],
                        lhsT=hTs[g % 3][:],
                        rhs=vb_sb[:, gv, j * 512 : (j + 1) * 512],
                        start=True,
                        stop=True,
                    )

            def epilogue_store(g):
                po = pos[g]
                o_sb = opool.tile([C, D_OUT], f16, tag="o")
                half = D_OUT // 2
                nc.scalar.copy(o_sb[:, 0:half], po[:, 0:half])
                nc.vector.tensor_copy(o_sb[:, half:], po[:, half:])
                nc.sync.dma_start(out=o_d[g], in_=o_sb[:])

            # depth-2 pipeline: PE runs two groups of mm1 ahead of each
            # mm2, so the DVE hcast + its semaphore handshake are fully
            # hidden behind PE work.
            depth = min(2, G - 1) if G > 1 else 0
            for g in range(depth):
                mm1(g)
                hcast(g)
            for g in range(depth, G):
                mm1(g)
                mm2(g - depth)
                hcast(g)
                epilogue_store(g - depth)
            for g in range(G - depth, G):
                mm2(g)
                epilogue_store(g)

    nc.compile()
    return nc


def _route(pids: np.ndarray, n_experts: int):
    """Group token indices by expert, chunk to MAX_CHUNK, assign to cores."""
    order = np.argsort(pids, kind="stable")
    counts = np.bincount(pids, minlength=n_experts)
    per_core = n_experts // N_CORES
    core_groups = [[] for _ in range(N_CORES)]
    off = 0
    for p in range(n_experts):
        toks = order[off : off + counts[p]]
        off += counts[p]
        for s in range(0, len(toks), MAX_CHUNK):
            core_groups[p // per_core].append((p, toks[s : s + MAX_CHUNK]))
    return core_groups


def _capacity(core_groups):
    G = max(len(gs) for gs in core_groups)
    maxlen = max((len(t) for gs in core_groups for _, t in gs), default=1)
    C = int(min(MAX_CHUNK, max(16, 4 * -(-maxlen // 4))))
    return G, C


def _pack_core(groups, G, C, x8, U8, V16, bias16):
    """Build one core's in_map from its (pid, token) groups."""
    FW = KC * C + KC * RANK
    xu = np.zeros((128, G, FW), F8)
    vb = np.zeros((RANK + 1, G, D_OUT), F16)
    for gi, (p, toks) in enumerate(groups):
        blk = np.zeros((C, D_IN), F8)
        blk[: len(toks)] = x8[toks]
        # [C, D] -> [d, t] -> [k, dp, t] -> [dp, k, t] -> flat [dp, k*t]
        xu[:, gi, : KC * C] = (
            blk.T.reshape(KC, 128, C).transpose(1, 0, 2).reshape(128, KC * C)
        )
        xu[:, gi, KC * C :] = (
            U8[p].reshape(KC, 128, RANK).transpose(1, 0, 2).reshape(128, KC * RANK)
        )
        vb[:RANK, gi] = V16[p]
        vb[RANK, gi] = bias16
    return {"xug": xu, "vbg": vb}


def _unpack(og_list, core_groups, N):
    out = np.zeros((N, D_OUT), np.float32)
    for c in range(N_CORES):
        og = np.asarray(og_list[c]).astype(np.float32)
        for gi, (p, toks) in enumerate(core_groups[c]):
            out[toks] = og[gi, : len(toks)]
    return out


def kernel(x, pids, U, V, bias):
    global LAST_EXEC_TIME_NS, LAST_RESULTS
    from concourse.bass_utils import run_bass_kernel_spmd

    x = np.asarray(x, dtype=np.float32)
    pids_np = np.asarray(pids).astype(np.int64)
    U = np.asarray(U, dtype=np.float32)
    V = np.asarray(V, dtype=np.float32)
    bias = np.asarray(bias, dtype=np.float32)

    N = x.shape[0]
    P = U.shape[0]

    x8 = x.astype(F8)
    U8 = U.astype(F8)
    V16 = V.astype(F16)
    bias16 = bias.astype(F16)

    core_groups = _route(pids_np, P)
    G, C = _capacity(core_groups)

    in_maps = [
        _pack_core(core_groups[c], G, C, x8, U8, V16, bias16)
        for c in range(N_CORES)
    ]

    key = (G, C)
    if key not in _PROGRAM_CACHE:
        _PROGRAM_CACHE[key] = _build_program(G, C)
    nc = _PROGRAM_CACHE[key]

    trace = os.environ.get("KERNEL_TRACE", "0") == "1"
    res = run_bass_kernel_spmd(nc, in_maps, list(range(N_CORES)), trace=trace)
    LAST_EXEC_TIME_NS = res.exec_time_ns
    LAST_RESULTS = res

    return _unpack([res.results[c]["og"] for c in range(N_CORES)], core_groups, N)
